# revision 1
# baseline (speedup 1.0000x reference)
"""Bass/Tile Trainium2 kernel for masked-bank BatchConv2D.

Math (matches the reference nn.Module):
    mask[o, j]   = j < connect_nums[o]                       (j in [0, 64))
    kdense[o, c] = sum_{j : j%32==c} weights[o, j] * mask[o, j]   -> [64, 32, 3, 3]
    out          = conv2d(x, kdense, VALID) + bias[None]          -> [B, 64, 126, 126]

Strategy: data-parallel over batch (8 cores x 4 images). Per core, conv is
computed as 3 accumulating matmuls (one per kernel-column dx) with the
contraction dim packed as (dy, c) = 96 partitions. Input-image chunks are
replicated on-chip into 3 row-shifted partition blocks (X3 tile, built by
one HBM load + two SBUF->SBUF shift DMAs) so a single [96, N] matmul rhs
covers all (dy, c). Output is produced in 4-row psum tiles (N = 504 fp32,
one bank); a vector add fuses the +bias with the psum->SBUF evacuation, and
stores are batched (32 output rows per DMA) to keep trigger overhead off the
queues. The kernel is DMA-bandwidth-bound: per-core HBM runs at ~220 GB/s
with all 8 cores active, so prefetch depth / queue spreading, not the PE,
set the wall time.

Modes (BASS_CONV_MODE): "f32r" (default) streams fp32 data through the PE's
single-pass FP32R mode (~tf32 precision, rel err ~2e-4, ~200 us); "bf16"
uses bf16 operands with PE column-pair tiling (two row-tiles concurrently,
psum halves 0:64/64:128; rel err ~2.6e-3, ~168 us); "f32" is exact fp32
with pair tiling (~230 us). f32r cannot column-pair (ISA: fmap/weights and
psum dst must start at partition 0), hence its serial layout.
"""

import os
import sys

for _p in ("/opt/trn_rl_repo",):
    if os.path.isdir(_p) and _p not in sys.path:
        sys.path.append(_p)

import numpy as np

# Problem dims (hardcoded per contract)
B, CIN, COUT = 32, 32, 64
H, W = 128, 128
KH = KW = 3
HO = WO = 126
MAXCN = 64
NCORES = 8
BL = B // NCORES  # local batch per core

# chunks of output rows per image: (x_row_start, n_x_rows, out_row_start, n_out_rows)
CHUNKS = [(0, 34, 0, 32), (32, 34, 32, 32), (64, 34, 64, 32), (96, 32, 96, 30)]
X3W = 34 * W  # x3 tile free size (elements)

_MODE = os.environ.get("BASS_CONV_MODE", "f32r")

_RUNNER_CACHE = {}


def _split_waits(nc, mybir, maxw=1):
    """This walrus build only accepts one sem-wait per instruction; hoist
    extra waits onto preceding NoOps on the same engine."""
    for f in nc.m.functions:
        for bb in f.blocks:
            newlist = []
            for inst in bb.instructions:
                si = inst.sync_info
                waits = list(si.on_wait) if si and si.on_wait else []
                if len(waits) > maxw:
                    chunks = [waits[i : i + maxw] for i in range(0, len(waits), maxw)]
                    for ci, ch in enumerate(chunks[:-1]):
                        nop = mybir.InstNoOp(
                            name=f"{inst.name}-ws{ci}", ins=[], outs=[]
                        )
                        nop.engine = inst.engine
                        nop.sync_info = mybir.SyncInfo(on_wait=list(ch), on_update=[])
                        newlist.append(nop)
                    si.on_wait = chunks[-1]
                newlist.append(inst)
            bb.instructions = newlist


def build_nc(mode=_MODE, split_waits=True):
    import concourse.bass as bass
    import concourse.mybir as mybir
    from concourse.tile import TileContext

    f32 = mybir.dt.float32
    i32 = mybir.dt.int32
    if mode == "bf16":
        mmdt = mybir.dt.bfloat16
    elif mode == "f32r":
        mmdt = mybir.dt.float32r
    else:
        mmdt = f32
    # storage dtype of matmul operand tiles: the BIR verifier requires fp32r
    # matmul operands to be *produced* as float32r, so the x3/lhsT tiles are
    # declared float32r and the copies into them perform the rounding.
    stdt = mmdt if mode in ("bf16", "f32r") else f32

    # f32r matmuls cannot target psum partitions 64:128 (ISA: dst partition
    # must be 0 for 4-byte non-exact modes), so f32r runs the "serial"
    # layout: one [64, N] psum tile at base 0 per output row-tile. bf16/f32
    # run the "paired" layout: two row-tiles concurrently via PE column
    # tiling (psum halves 0:64 / 64:128).
    paired = mode != "f32r"

    nc = bass.Bass()
    xs = nc.declare_dram_parameter("xs", [BL, CIN, H, W], f32, isOutput=False)
    wt = nc.declare_dram_parameter("wt", [COUT, MAXCN * 9], f32, isOutput=False)
    bias = nc.declare_dram_parameter("bias", [COUT, HO * WO], f32, isOutput=False)
    cn = nc.declare_dram_parameter("cn", [COUT, 1], i32, isOutput=False)
    iota = nc.declare_dram_parameter("iota", [COUT, MAXCN], f32, isOutput=False)
    ident = nc.declare_dram_parameter("ident", [COUT, COUT], f32, isOutput=False)
    out = nc.declare_dram_parameter("out", [BL, COUT, HO * WO], f32, isOutput=True)

    def as_mm(ap):
        return ap

    # queue-mode SBUF allocator: freed prep-pool space is not immediately
    # reused by the x3 pool, so x3 loads don't inherit a WAR dependency on
    # the weight-prep chain
    with TileContext(nc, pool_alloc_mode="queue") as tc:
        with tc.tile_pool(name="const", bufs=1) as constp:
            # persistent tiles
            bias2 = constp.tile([128 if paired else 64, HO * WO], f32)
            lhsT = constp.tile([96, 3 * COUT], stdt)

            # ---- weight prep ----
            with (
                tc.tile_pool(name="prep", bufs=1) as prepp,
                tc.tile_pool(name="tps", bufs=3, space="PSUM") as tpsp,
            ):
                prep_dmas = []
                w_sb = prepp.tile([COUT, MAXCN * 9], f32)
                prep_dmas.append(nc.sync.dma_start(out=w_sb[:], in_=wt[:]))
                cn_i = prepp.tile([COUT, 1], i32)
                prep_dmas.append(nc.sync.dma_start(out=cn_i[:], in_=cn[:]))
                iota_sb = prepp.tile([COUT, MAXCN], f32)
                prep_dmas.append(nc.sync.dma_start(out=iota_sb[:], in_=iota[:]))
                ident_sb = prepp.tile([COUT, COUT], f32)
                prep_dmas.append(nc.sync.dma_start(out=ident_sb[:], in_=ident[:]))

                cn_f = prepp.tile([COUT, 1], f32)
                nc.vector.tensor_copy(out=cn_f[:], in_=cn_i[:])
                mask = prepp.tile([COUT, MAXCN], f32)
                # mask[o, j] = (j < cn[o]) -> 1.0 / 0.0
                nc.vector.tensor_scalar(
                    out=mask[:],
                    in0=iota_sb[:],
                    scalar1=cn_f[:],
                    scalar2=None,
                    op0=mybir.AluOpType.is_lt,
                )
                # replicate mask over the 9 (dy,dx) slots: mask9[o, j*9+k]
                mask9 = prepp.tile([COUT, MAXCN * 9], f32)
                m9v = mask9.rearrange("p (j k) -> p j k", k=9)
                for k in range(9):
                    nc.vector.tensor_copy(out=m9v[:, :, k], in_=mask[:])
                wm = prepp.tile([COUT, MAXCN * 9], f32)
                nc.vector.tensor_mul(out=wm[:], in0=w_sb[:], in1=mask9[:])
                # fold j and j+32 (same input channel): kd[o, (c, dy, dx)]
                kd = prepp.tile([COUT, CIN * 9], f32)
                nc.vector.tensor_add(
                    out=kd[:], in0=wm[:, 0 : CIN * 9], in1=wm[:, CIN * 9 : MAXCN * 9]
                )
                # reorder to (dx, dy, c) contiguous, then transpose per dx:
                # [64, (dy, c)] -> [96, 64]
                kd4 = kd.rearrange("p (c dy dx) -> p dx dy c", c=CIN, dy=3, dx=3)
                kdr = prepp.tile([COUT, CIN * 9], f32)
                kdr4 = kdr.rearrange("p (dx dy c) -> p dx dy c", c=CIN, dy=3, dx=3)
                for dx in range(3):
                    nc.vector.tensor_copy(out=kdr4[:, dx], in_=kd4[:, dx])
                for dx in range(3):
                    tp = tpsp.tile([96, COUT], f32)
                    nc.tensor.transpose(
                        out=tp[:],
                        in_=kdr[:, dx * 96 : (dx + 1) * 96],
                        identity=ident_sb[:],
                    )
                    nc.vector.tensor_copy(
                        out=lhsT[:, dx * COUT : (dx + 1) * COUT], in_=tp[:]
                    )

            # ---- main loop (software-pipelined chunks) ----
            chunks = [
                (bi, r0, nxr, oy0, nor)
                for bi in range(BL)
                for (r0, nxr, oy0, nor) in CHUNKS
            ]
            with (
                tc.tile_pool(name="x3", bufs=4) as x3p,
                tc.tile_pool(name="ps", bufs=8 if not paired else 6, space="PSUM") as psp,
                tc.tile_pool(name="ob", bufs=2) as obp,
            ):
                x3_tiles = {}

                from concourse.tile_rust import add_dep_helper

                def issue_x3(ci):
                    bi, r0, nxr, oy0, nor = chunks[ci]
                    x3 = x3p.tile([96, X3W], stdt, tag="x3", name=f"x3_{ci}")
                    if mode == "bf16":
                        ld = nc.gpsimd.dma_start(
                            out=x3[0:32, 0 : nxr * W], in_=xs[bi, :, r0 : r0 + nxr, :]
                        )
                    else:
                        xin = xs[bi, :, r0 : r0 + nxr, :]
                        if mode == "f32r":
                            xin = xin.bitcast(mmdt)
                        ld = nc.sync.dma_start(out=x3[0:32, 0 : nxr * W], in_=xin)

                    nc.scalar.dma_start(
                        out=x3[32:64, 0 : nor * W],
                        in_=x3[0:32, W : (nor + 1) * W],
                    )
                    nc.gpsimd.dma_start(
                        out=x3[64:96, 0 : nor * W],
                        in_=x3[0:32, 2 * W : (nor + 2) * W],
                    )
                    x3_tiles[ci] = x3

                issue_x3(0)
                issue_x3(1)

                # bias is loaded piecewise during the first batch's chunks so
                # its 4 MB doesn't flood the DMA fabric at startup. Upper half
                # (paired only): shifted by 4 output rows so one [128, N]
                # vector add covers a psum pair.
                SH = 4 * WO

                def issue_bias(ci):
                    _, _, _, oy0, nor = chunks[ci]
                    a, b = oy0 * WO, (oy0 + nor) * WO
                    nc.gpsimd.dma_start(out=bias2[0:64, a:b], in_=bias[:, a:b])
                    if paired:
                        hb = min(b - SH, HO * WO - SH)
                        nc.gpsimd.dma_start(
                            out=bias2[64:128, a - SH if a >= SH else 0 : hb],
                            in_=bias[:, max(a, SH) : hb + SH],
                        )

                for ci, (bi, r0, nxr, oy0, nor) in enumerate(chunks):
                    if ci < len(CHUNKS):
                        issue_bias(ci)
                    if ci + 2 < len(chunks):
                        issue_x3(ci + 2)
                    x3 = x3_tiles.pop(ci)
                    x3v = x3.rearrange("p (r c) -> p r c", c=W)
                    if True:

                        if paired:
                            # pairs of two 4-row tiles (slots A/B via PE col
                            # tiling); dx-outer over groups of 3 pairs so the
                            # stationary weights only change every 6 matmuls
                            npairs = nor // 8
                            rem = nor - npairs * 8  # 0 or 6
                            allp = npairs + (1 if rem else 0)
                            p0 = 0
                            while p0 < allp:
                                g = min(4, allp - p0)
                                pss = []
                                for pi in range(p0, p0 + g):
                                    nr = 4 if pi < npairs else rem // 2
                                    ps = psp.tile([128, 4 * WO], f32, tag="ps")
                                    pss.append((pi, nr, ps))
                                for dx in range(3):
                                    lw = as_mm(lhsT[:, dx * COUT : (dx + 1) * COUT])
                                    for (pi, nr, ps) in pss:
                                        yl = 8 * pi
                                        N = nr * WO
                                        nc.tensor.matmul(
                                            ps[0:64, 0:N],
                                            lhsT=lw,
                                            rhs=as_mm(
                                                x3v[:, yl : yl + nr, dx : dx + WO]
                                            ),
                                            start=(dx == 0),
                                            stop=(dx == 2),
                                            skip_group_check=True,
                                        )
                                        nc.tensor.matmul(
                                            ps[64:128, 0:N],
                                            lhsT=lw,
                                            rhs=as_mm(
                                                x3v[
                                                    :,
                                                    yl + nr : yl + 2 * nr,
                                                    dx : dx + WO,
                                                ]
                                            ),
                                            start=(dx == 0),
                                            stop=(dx == 2),
                                            skip_group_check=True,
                                        )
                                ob = obp.tile([128, 4 * 4 * WO], f32, tag="ob")
                                for gi, (pi, nr, ps) in enumerate(pss):
                                    yg = oy0 + 8 * pi
                                    N = nr * WO
                                    o0 = gi * 4 * WO
                                    if nr == 4:
                                        nc.vector.tensor_add(
                                            out=ob[:, o0 : o0 + N],
                                            in0=ps[:, 0:N],
                                            in1=bias2[:, yg * WO : yg * WO + N],
                                        )
                                    else:
                                        nc.vector.tensor_add(
                                            out=ob[0:64, o0 : o0 + N],
                                            in0=ps[0:64, 0:N],
                                            in1=bias2[0:64, yg * WO : yg * WO + N],
                                        )
                                        nc.vector.tensor_add(
                                            out=ob[64:128, o0 : o0 + N],
                                            in0=ps[64:128, 0:N],
                                            in1=bias2[
                                                64:128,
                                                (yg + nr) * WO
                                                - SH : (yg + nr) * WO
                                                - SH
                                                + N,
                                            ],
                                        )
                                # one batched store per group half; slot-A
                                # rows sit at even 4-row blocks, slot-B at odd
                                fullg = all(nr == 4 for (_, nr, _) in pss)
                                yg0 = oy0 + 8 * p0
                                if fullg:
                                    dv = out[
                                        bi, :, yg0 * WO : (yg0 + 8 * g) * WO
                                    ].rearrange(
                                        "o (pb h n) -> o pb h n", h=2, n=4 * WO
                                    )
                                    obv = ob.rearrange("p (t n) -> p t n", n=4 * WO)
                                    nc.scalar.dma_start(
                                        out=dv[:, :, 0, :], in_=obv[0:64, 0:g]
                                    )
                                    nc.scalar.dma_start(
                                        out=dv[:, :, 1, :], in_=obv[64:128, 0:g]
                                    )
                                else:
                                    for gi, (pi, nr, ps) in enumerate(pss):
                                        yg = oy0 + 8 * pi
                                        N = nr * WO
                                        o0 = gi * 4 * WO
                                        nc.scalar.dma_start(
                                            out=out[bi, :, yg * WO : yg * WO + N],
                                            in_=ob[0:64, o0 : o0 + N],
                                        )
                                        nc.scalar.dma_start(
                                            out=out[
                                                bi,
                                                :,
                                                (yg + nr) * WO : (yg + nr) * WO + N,
                                            ],
                                            in_=ob[64:128, o0 : o0 + N],
                                        )
                                p0 += g
                        else:
                            # serial layout: 4-row tiles, dx-outer over groups
                            # of 6 tiles (weights change every 6 matmuls)
                            ntiles = (nor + 3) // 4
                            t0 = 0
                            while t0 < ntiles:
                                g = min(8, ntiles - t0)
                                tss = []
                                for ti in range(t0, t0 + g):
                                    nr = min(4, nor - 4 * ti)
                                    ps = psp.tile([64, 4 * WO], f32, tag="ps")
                                    tss.append((ti, nr, ps))
                                for dx in range(3):
                                    lw = lhsT[:, dx * COUT : (dx + 1) * COUT]
                                    for (ti, nr, ps) in tss:
                                        yl = 4 * ti
                                        nc.tensor.matmul(
                                            ps[:, 0 : nr * WO],
                                            lhsT=lw,
                                            rhs=x3v[:, yl : yl + nr, dx : dx + WO],
                                            start=(dx == 0),
                                            stop=(dx == 2),
                                            skip_group_check=True,
                                        )
                                ob = obp.tile([64, 8 * 4 * WO], f32, tag="ob")
                                rows = 0
                                for gi, (ti, nr, ps) in enumerate(tss):
                                    yg = oy0 + 4 * ti
                                    N = nr * WO
                                    o0 = gi * 4 * WO
                                    nc.vector.tensor_add(
                                        out=ob[:, o0 : o0 + N],
                                        in0=ps[:, 0:N],
                                        in1=bias2[0:64, yg * WO : yg * WO + N],
                                    )
                                    rows += nr
                                yg0 = oy0 + 4 * t0
                                if rows == 4 * g:
                                    nc.scalar.dma_start(
                                        out=out[
                                            bi, :, yg0 * WO : (yg0 + rows) * WO
                                        ],
                                        in_=ob[:, 0 : rows * WO],
                                    )
                                else:
                                    # ragged tail: last tile shorter; store
                                    # full tiles in one DMA, tail separately
                                    nf = rows - tss[-1][1]
                                    nc.scalar.dma_start(
                                        out=out[bi, :, yg0 * WO : (yg0 + nf) * WO],
                                        in_=ob[:, 0 : nf * WO],
                                    )
                                    nc.scalar.dma_start(
                                        out=out[
                                            bi,
                                            :,
                                            (yg0 + nf) * WO : (yg0 + rows) * WO,
                                        ],
                                        in_=ob[
                                            :,
                                            (g - 1) * 4 * WO : (g - 1) * 4 * WO
                                            + tss[-1][1] * WO,
                                        ],
                                    )
                                t0 += g

    if split_waits:
        _split_waits(nc, mybir)
    return nc


def _make_inputs(x, weights, bias, connect_nums):
    """Host-side reshapes only (no input-dependent compute)."""
    x = np.ascontiguousarray(np.asarray(x, dtype=np.float32))
    w = np.ascontiguousarray(
        np.asarray(weights, dtype=np.float32).reshape(COUT, MAXCN * 9)
    )
    b = np.ascontiguousarray(np.asarray(bias, dtype=np.float32).reshape(COUT, HO * WO))
    cnv = np.ascontiguousarray(
        np.asarray(connect_nums, dtype=np.int32).reshape(COUT, 1)
    )
    iota = np.ascontiguousarray(
        np.tile(np.arange(MAXCN, dtype=np.float32), (COUT, 1))
    )
    ident = np.eye(COUT, dtype=np.float32)
    shards = x.reshape(NCORES, BL, CIN, H, W)
    in_maps = [
        {
            "xs": shards[c],
            "wt": w,
            "bias": b,
            "cn": cnv,
            "iota": iota,
            "ident": ident,
        }
        for c in range(NCORES)
    ]
    return in_maps


def _get_runner(mode=_MODE):
    """Build + jit once; reuse across kernel() calls."""
    if mode in _RUNNER_CACHE:
        return _RUNNER_CACHE[mode]

    import jax
    from jax.experimental.shard_map import shard_map
    from jax.sharding import Mesh, PartitionSpec

    import concourse.mybir as mybir
    from concourse.bass2jax import (
        _bass_exec_p,
        install_neuronx_cc_hook,
        partition_id_tensor,
    )

    nc = build_nc(mode)
    install_neuronx_cc_hook()

    partition_name = nc.partition_id_tensor.name if nc.partition_id_tensor else None
    in_names = []
    out_names = []
    out_avals = []
    zero_shapes = []
    for alloc in nc.m.functions[0].allocations:
        if not isinstance(alloc, mybir.MemoryLocationSet):
            continue
        name = alloc.memorylocations[0].name
        if alloc.kind == "ExternalInput":
            if name != partition_name:
                in_names.append(name)
        elif alloc.kind == "ExternalOutput":
            out_names.append(name)
            shape = tuple(alloc.tensor_shape)
            dtype = mybir.dt.np(alloc.dtype)
            out_avals.append(jax.core.ShapedArray(shape, dtype))
            zero_shapes.append((shape, dtype))
    n_params = len(in_names)
    n_outs = len(out_names)
    all_names = in_names + out_names
    if partition_name is not None:
        all_names = all_names + [partition_name]

    def _body(*args):
        operands = list(args)
        if partition_name is not None:
            operands.append(partition_id_tensor())
        outs = _bass_exec_p.bind(
            *operands,
            out_avals=tuple(out_avals),
            in_names=tuple(all_names),
            out_names=tuple(out_names),
            lowering_input_output_aliases=(),
            sim_require_finite=True,
            sim_require_nnan=True,
            nc=nc,
        )
        return tuple(outs)

    devices = jax.devices()[:NCORES]
    mesh = Mesh(np.asarray(devices), ("core",))
    in_specs = (PartitionSpec("core"),) * (n_params + n_outs)
    out_specs = (PartitionSpec("core"),) * n_outs
    sharded = jax.jit(
        shard_map(
            _body, mesh=mesh, in_specs=in_specs, out_specs=out_specs, check_rep=False
        ),
        donate_argnums=tuple(range(n_params, n_params + n_outs)),
        keep_unused=True,
    )

    def run(in_maps):
        concat_in = [
            np.concatenate([np.asarray(in_maps[c][nm]) for c in range(NCORES)], axis=0)
            for nm in in_names
        ]
        concat_zeros = [
            np.zeros((NCORES * s[0],) + tuple(s[1:]), dt) for (s, dt) in zero_shapes
        ]
        out_arrs = sharded(*concat_in, *concat_zeros)
        outv = np.asarray(out_arrs[0])  # single output "out"
        return outv.reshape(NCORES, BL, COUT, HO, WO)

    _RUNNER_CACHE[mode] = run
    return run


def kernel(x, weights, bias, connect_nums):
    run = _get_runner()
    in_maps = _make_inputs(x, weights, bias, connect_nums)
    outs = run(in_maps)
    return np.ascontiguousarray(outs.reshape(B, COUT, HO, WO))



# revision 4
# speedup vs baseline: 1.1585x; 1.1585x over previous
"""Bass/Tile Trainium2 kernel for masked-bank BatchConv2D.

Math (matches the reference nn.Module):
    mask[o, j]   = j < connect_nums[o]                       (j in [0, 64))
    kdense[o, c] = sum_{j : j%32==c} weights[o, j] * mask[o, j]   -> [64, 32, 3, 3]
    out          = conv2d(x, kdense, VALID) + bias[None]          -> [B, 64, 126, 126]

Strategy: data-parallel over batch (8 cores x 4 images). Per core, conv is
computed as 3 accumulating matmuls (one per kernel-column dx) with the
contraction dim packed as (dy, c) = 96 partitions. Input-image chunks are
replicated on-chip into 3 row-shifted partition blocks (X3 tile, built by
one HBM load + two SBUF->SBUF shift DMAs).

The kernel is SDMA-engine bound (16 engines/core, ~27 GB/s each), so the
main loop minimizes bytes through the DMA fabric:
  - operands and PE stream in bf16 (PE column-pair tiling: psum lower half
    = image A, upper half = image B, same output rows -> both halves share
    identical bias rows and stores are fully contiguous per partition)
  - bias is read from HBM once ([64, HO*WO] f32) and duplicated to
    partitions 64:128 by an on-chip SBUF->SBUF copy (fabric, not HBM)
  - outputs are stored as bf16 (the rounding happens on-device in the DVE
    psum-evacuation add; the host only widens bf16->f32, which is exact)
  - stores are one DMA per (image, 32-row chunk): [64 part, 8 KB contig]

Modes (BASS_CONV_MODE): "bf16" (default) as above; "f32" exact fp32 with
the same image-paired layout (f32 stores); "f32r" streams fp32 through the
PE's single-pass FP32R mode (~tf32, rel err ~2e-4) with a serial 64-part
psum layout (ISA: f32r matmul dst must start at partition 0, so no column
pairing).
"""

import os
import sys

for _p in ("/opt/trn_rl_repo",):
    if os.path.isdir(_p) and _p not in sys.path:
        sys.path.append(_p)

import numpy as np

# Problem dims (hardcoded per contract)
B, CIN, COUT = 32, 32, 64
H, W = 128, 128
KH = KW = 3
HO = WO = 126
MAXCN = 64
NCORES = 8
BL = B // NCORES  # local batch per core

# chunks of output rows per image: (x_row_start, n_x_rows, out_row_start, n_out_rows)
CHUNKS = [(0, 34, 0, 32), (32, 34, 32, 32), (64, 34, 64, 32), (96, 32, 96, 30)]
X3W = 34 * W  # x3 tile free size (elements)

_MODE = os.environ.get("BASS_CONV_MODE", "bf16")

_RUNNER_CACHE = {}


def _split_waits(nc, mybir, maxw=1):
    """This walrus build only accepts one sem-wait per instruction; hoist
    extra waits onto preceding NoOps on the same engine."""
    for f in nc.m.functions:
        for bb in f.blocks:
            newlist = []
            for inst in bb.instructions:
                si = inst.sync_info
                waits = list(si.on_wait) if si and si.on_wait else []
                if len(waits) > maxw:
                    chunks = [waits[i : i + maxw] for i in range(0, len(waits), maxw)]
                    for ci, ch in enumerate(chunks[:-1]):
                        nop = mybir.InstNoOp(
                            name=f"{inst.name}-ws{ci}", ins=[], outs=[]
                        )
                        nop.engine = inst.engine
                        nop.sync_info = mybir.SyncInfo(on_wait=list(ch), on_update=[])
                        newlist.append(nop)
                    si.on_wait = chunks[-1]
                newlist.append(inst)
            bb.instructions = newlist


def build_nc(mode=_MODE, split_waits=True):
    import concourse.bass as bass
    import concourse.mybir as mybir
    from concourse.tile import TileContext

    f32 = mybir.dt.float32
    i32 = mybir.dt.int32
    if mode == "bf16":
        mmdt = mybir.dt.bfloat16
    elif mode == "f32r":
        mmdt = mybir.dt.float32r
    else:
        mmdt = f32
    # storage dtype of matmul operand tiles: the BIR verifier requires fp32r
    # matmul operands to be *produced* as float32r, so the x3/lhsT tiles are
    # declared float32r and the copies into them perform the rounding.
    stdt = mmdt if mode in ("bf16", "f32r") else f32
    # DRAM output dtype: bf16 mode stores rounded outputs (host widens).
    outdt = mybir.dt.bfloat16 if mode == "bf16" else f32

    # f32r matmuls cannot target psum partitions 64:128 (ISA: dst partition
    # must be 0 for 4-byte non-exact modes), so f32r runs the "serial"
    # layout: one [64, N] psum tile at base 0 per output row-tile. bf16/f32
    # run the "paired" layout: two images concurrently via PE column
    # tiling (psum halves 0:64 / 64:128, same output rows).
    paired = mode != "f32r"

    nc = bass.Bass()
    xs = nc.declare_dram_parameter("xs", [BL, CIN, H, W], f32, isOutput=False)
    wt = nc.declare_dram_parameter("wt", [COUT, MAXCN * 9], f32, isOutput=False)
    bias = nc.declare_dram_parameter("bias", [COUT, HO * WO], f32, isOutput=False)
    cn = nc.declare_dram_parameter("cn", [COUT, 1], i32, isOutput=False)
    iota = nc.declare_dram_parameter("iota", [COUT, MAXCN], f32, isOutput=False)
    ident = nc.declare_dram_parameter("ident", [COUT, COUT], f32, isOutput=False)
    out = nc.declare_dram_parameter("out", [BL, COUT, HO * WO], outdt, isOutput=True)

    # queue-mode SBUF allocator: freed prep-pool space is not immediately
    # reused by the x3 pool, so x3 loads don't inherit a WAR dependency on
    # the weight-prep chain
    with TileContext(nc, pool_alloc_mode="queue") as tc:
        with tc.tile_pool(name="const", bufs=1) as constp:
            # persistent tiles
            bias2 = constp.tile([128 if paired else 64, HO * WO], f32)
            lhsT = constp.tile([96, 3 * COUT], stdt)

            # ---- weight prep ----
            with (
                tc.tile_pool(name="prep", bufs=1) as prepp,
                tc.tile_pool(name="tps", bufs=3, space="PSUM") as tpsp,
            ):
                prep_dmas = []
                w_sb = prepp.tile([COUT, MAXCN * 9], f32)
                prep_dmas.append(nc.sync.dma_start(out=w_sb[:], in_=wt[:]))
                cn_i = prepp.tile([COUT, 1], i32)
                prep_dmas.append(nc.sync.dma_start(out=cn_i[:], in_=cn[:]))
                iota_sb = prepp.tile([COUT, MAXCN], f32)
                prep_dmas.append(nc.sync.dma_start(out=iota_sb[:], in_=iota[:]))
                ident_sb = prepp.tile([COUT, COUT], f32)
                prep_dmas.append(nc.sync.dma_start(out=ident_sb[:], in_=ident[:]))

                cn_f = prepp.tile([COUT, 1], f32)
                nc.vector.tensor_copy(out=cn_f[:], in_=cn_i[:])
                mask = prepp.tile([COUT, MAXCN], f32)
                # mask[o, j] = (j < cn[o]) -> 1.0 / 0.0
                nc.vector.tensor_scalar(
                    out=mask[:],
                    in0=iota_sb[:],
                    scalar1=cn_f[:],
                    scalar2=None,
                    op0=mybir.AluOpType.is_lt,
                )
                # replicate mask over the 9 (dy,dx) slots: mask9[o, j*9+k]
                mask9 = prepp.tile([COUT, MAXCN * 9], f32)
                m9v = mask9.rearrange("p (j k) -> p j k", k=9)
                for k in range(9):
                    nc.vector.tensor_copy(out=m9v[:, :, k], in_=mask[:])
                wm = prepp.tile([COUT, MAXCN * 9], f32)
                nc.vector.tensor_mul(out=wm[:], in0=w_sb[:], in1=mask9[:])
                # fold j and j+32 (same input channel): kd[o, (c, dy, dx)]
                kd = prepp.tile([COUT, CIN * 9], f32)
                nc.vector.tensor_add(
                    out=kd[:], in0=wm[:, 0 : CIN * 9], in1=wm[:, CIN * 9 : MAXCN * 9]
                )
                # reorder to (dx, dy, c) contiguous, then transpose per dx:
                # [64, (dy, c)] -> [96, 64]
                kd4 = kd.rearrange("p (c dy dx) -> p dx dy c", c=CIN, dy=3, dx=3)
                kdr = prepp.tile([COUT, CIN * 9], f32)
                kdr4 = kdr.rearrange("p (dx dy c) -> p dx dy c", c=CIN, dy=3, dx=3)
                for dx in range(3):
                    nc.vector.tensor_copy(out=kdr4[:, dx], in_=kd4[:, dx])
                for dx in range(3):
                    tp = tpsp.tile([96, COUT], f32)
                    nc.tensor.transpose(
                        out=tp[:],
                        in_=kdr[:, dx * 96 : (dx + 1) * 96],
                        identity=ident_sb[:],
                    )
                    nc.vector.tensor_copy(
                        out=lhsT[:, dx * COUT : (dx + 1) * COUT], in_=tp[:]
                    )

            if paired:
                # ---- main loop: image-paired column tiling ----
                # macro-steps: (image pair, row chunk); psum lower half is
                # image 2*pr, upper half image 2*pr+1, same output rows.
                msteps = [
                    (2 * pr, r0, nxr, oy0, nor)
                    for pr in range(BL // 2)
                    for (r0, nxr, oy0, nor) in CHUNKS
                ]
                with (
                    tc.tile_pool(name="x3", bufs=6) as x3p,
                    tc.tile_pool(name="ps", bufs=6, space="PSUM") as psp,
                    tc.tile_pool(name="ob", bufs=2) as obp,
                ):
                    x3_tiles = {}

                    def issue_x3(mi):
                        biA, r0, nxr, oy0, nor = msteps[mi]
                        pair = []
                        for k, eng1, eng2 in ((0, nc.sync, nc.scalar),
                                              (1, nc.scalar, nc.sync)):
                            x3 = x3p.tile(
                                [96, X3W], stdt, tag=f"x3{k}", name=f"x3_{mi}_{k}"
                            )
                            if mode == "bf16":
                                # dtype-cast load must ride SWDGE (gpsimd)
                                nc.gpsimd.dma_start(
                                    out=x3[0:32, 0 : nxr * W],
                                    in_=xs[biA + k, :, r0 : r0 + nxr, :],
                                )
                            else:
                                nc.sync.dma_start(
                                    out=x3[0:32, 0 : nxr * W],
                                    in_=xs[biA + k, :, r0 : r0 + nxr, :],
                                )
                            eng1.dma_start(
                                out=x3[32:64, 0 : nor * W],
                                in_=x3[0:32, W : (nor + 1) * W],
                            )
                            eng2.dma_start(
                                out=x3[64:96, 0 : nor * W],
                                in_=x3[0:32, 2 * W : (nor + 2) * W],
                            )
                            pair.append(x3)
                        x3_tiles[mi] = pair

                    issue_x3(0)
                    issue_x3(1)

                    # bias is loaded piecewise during the first image-pair's
                    # chunks so its 4 MB doesn't flood the DMA fabric at
                    # startup. Upper half (partitions 64:128) is an on-chip
                    # duplicate of the lower half (both psum halves cover the
                    # same output rows), built by SBUF->SBUF fabric DMA.
                    def issue_bias(ci):
                        _, _, oy0, nor = CHUNKS[ci]
                        a, b = oy0 * WO, (oy0 + nor) * WO
                        nc.sync.dma_start(out=bias2[0:64, a:b], in_=bias[:, a:b])
                        nc.sync.dma_start(
                            out=bias2[64:128, a:b], in_=bias2[0:64, a:b]
                        )

                    for mi, (biA, r0, nxr, oy0, nor) in enumerate(msteps):
                        if mi < len(CHUNKS):
                            issue_bias(mi)
                        if mi + 2 < len(msteps):
                            issue_x3(mi + 2)
                        x3A, x3B = x3_tiles.pop(mi)
                        xvA = x3A.rearrange("p (r c) -> p r c", c=W)
                        xvB = x3B.rearrange("p (r c) -> p r c", c=W)

                        # row-tiles of <=4 output rows; same tile index on
                        # both psum halves (image A lower, image B upper)
                        ntiles = (nor + 3) // 4
                        ob = obp.tile([128, 8 * 4 * WO], outdt, tag="ob")
                        t0 = 0
                        while t0 < ntiles:
                            g = min(4, ntiles - t0)
                            tss = []
                            for ti in range(t0, t0 + g):
                                nr = min(4, nor - 4 * ti)
                                ps = psp.tile([128, 4 * WO], f32, tag="ps")
                                tss.append((ti, nr, ps))
                            for dx in range(3):
                                lw = lhsT[:, dx * COUT : (dx + 1) * COUT]
                                for (ti, nr, ps) in tss:
                                    yl = 4 * ti
                                    N = nr * WO
                                    nc.tensor.matmul(
                                        ps[0:64, 0:N],
                                        lhsT=lw,
                                        rhs=xvA[:, yl : yl + nr, dx : dx + WO],
                                        start=(dx == 0),
                                        stop=(dx == 2),
                                        skip_group_check=True,
                                    )
                                    nc.tensor.matmul(
                                        ps[64:128, 0:N],
                                        lhsT=lw,
                                        rhs=xvB[:, yl : yl + nr, dx : dx + WO],
                                        start=(dx == 0),
                                        stop=(dx == 2),
                                        skip_group_check=True,
                                    )
                            # fused +bias psum evacuation; both halves read
                            # the same bias rows (upper = on-chip duplicate)
                            for (ti, nr, ps) in tss:
                                yg = oy0 + 4 * ti
                                N = nr * WO
                                nc.vector.tensor_add(
                                    out=ob[:, 4 * ti * WO : 4 * ti * WO + N],
                                    in0=ps[:, 0:N],
                                    in1=bias2[:, yg * WO : yg * WO + N],
                                )
                            t0 += g
                        # one contiguous store per image: [64, nor*WO]
                        nc.scalar.dma_start(
                            out=out[biA, :, oy0 * WO : (oy0 + nor) * WO],
                            in_=ob[0:64, 0 : nor * WO],
                        )
                        nc.scalar.dma_start(
                            out=out[biA + 1, :, oy0 * WO : (oy0 + nor) * WO],
                            in_=ob[64:128, 0 : nor * WO],
                        )
            else:
                # ---- main loop: serial layout (f32r) ----
                chunks = [
                    (bi, r0, nxr, oy0, nor)
                    for bi in range(BL)
                    for (r0, nxr, oy0, nor) in CHUNKS
                ]
                with (
                    tc.tile_pool(name="x3", bufs=4) as x3p,
                    tc.tile_pool(name="ps", bufs=8, space="PSUM") as psp,
                    tc.tile_pool(name="ob", bufs=2) as obp,
                ):
                    x3_tiles = {}

                    def issue_x3s(ci):
                        bi, r0, nxr, oy0, nor = chunks[ci]
                        x3 = x3p.tile([96, X3W], stdt, tag="x3", name=f"x3_{ci}")
                        xin = xs[bi, :, r0 : r0 + nxr, :]
                        if mode == "f32r":
                            xin = xin.bitcast(mmdt)
                        nc.sync.dma_start(out=x3[0:32, 0 : nxr * W], in_=xin)
                        nc.scalar.dma_start(
                            out=x3[32:64, 0 : nor * W],
                            in_=x3[0:32, W : (nor + 1) * W],
                        )
                        nc.gpsimd.dma_start(
                            out=x3[64:96, 0 : nor * W],
                            in_=x3[0:32, 2 * W : (nor + 2) * W],
                        )
                        x3_tiles[ci] = x3

                    issue_x3s(0)
                    issue_x3s(1)

                    def issue_bias_s(ci):
                        _, _, _, oy0, nor = chunks[ci]
                        a, b = oy0 * WO, (oy0 + nor) * WO
                        nc.gpsimd.dma_start(out=bias2[0:64, a:b], in_=bias[:, a:b])

                    for ci, (bi, r0, nxr, oy0, nor) in enumerate(chunks):
                        if ci < len(CHUNKS):
                            issue_bias_s(ci)
                        if ci + 2 < len(chunks):
                            issue_x3s(ci + 2)
                        x3 = x3_tiles.pop(ci)
                        x3v = x3.rearrange("p (r c) -> p r c", c=W)
                        # serial layout: 4-row tiles, dx-outer over groups
                        # of 8 tiles (weights change every 8 matmuls)
                        ntiles = (nor + 3) // 4
                        t0 = 0
                        while t0 < ntiles:
                            g = min(8, ntiles - t0)
                            tss = []
                            for ti in range(t0, t0 + g):
                                nr = min(4, nor - 4 * ti)
                                ps = psp.tile([64, 4 * WO], f32, tag="ps")
                                tss.append((ti, nr, ps))
                            for dx in range(3):
                                lw = lhsT[:, dx * COUT : (dx + 1) * COUT]
                                for (ti, nr, ps) in tss:
                                    yl = 4 * ti
                                    nc.tensor.matmul(
                                        ps[:, 0 : nr * WO],
                                        lhsT=lw,
                                        rhs=x3v[:, yl : yl + nr, dx : dx + WO],
                                        start=(dx == 0),
                                        stop=(dx == 2),
                                        skip_group_check=True,
                                    )
                            ob = obp.tile([64, 8 * 4 * WO], f32, tag="ob")
                            rows = 0
                            for gi, (ti, nr, ps) in enumerate(tss):
                                yg = oy0 + 4 * ti
                                N = nr * WO
                                o0 = gi * 4 * WO
                                nc.vector.tensor_add(
                                    out=ob[:, o0 : o0 + N],
                                    in0=ps[:, 0:N],
                                    in1=bias2[0:64, yg * WO : yg * WO + N],
                                )
                                rows += nr
                            yg0 = oy0 + 4 * t0
                            if rows == 4 * g:
                                nc.scalar.dma_start(
                                    out=out[bi, :, yg0 * WO : (yg0 + rows) * WO],
                                    in_=ob[:, 0 : rows * WO],
                                )
                            else:
                                # ragged tail: last tile shorter; store
                                # full tiles in one DMA, tail separately
                                nf = rows - tss[-1][1]
                                nc.scalar.dma_start(
                                    out=out[bi, :, yg0 * WO : (yg0 + nf) * WO],
                                    in_=ob[:, 0 : nf * WO],
                                )
                                nc.scalar.dma_start(
                                    out=out[
                                        bi,
                                        :,
                                        (yg0 + nf) * WO : (yg0 + rows) * WO,
                                    ],
                                    in_=ob[
                                        :,
                                        (g - 1) * 4 * WO : (g - 1) * 4 * WO
                                        + tss[-1][1] * WO,
                                    ],
                                )
                            t0 += g

    if split_waits:
        _split_waits(nc, mybir)
    return nc


def _make_inputs(x, weights, bias, connect_nums):
    """Host-side reshapes only (no input-dependent compute)."""
    x = np.ascontiguousarray(np.asarray(x, dtype=np.float32))
    w = np.ascontiguousarray(
        np.asarray(weights, dtype=np.float32).reshape(COUT, MAXCN * 9)
    )
    b = np.ascontiguousarray(np.asarray(bias, dtype=np.float32).reshape(COUT, HO * WO))
    cnv = np.ascontiguousarray(
        np.asarray(connect_nums, dtype=np.int32).reshape(COUT, 1)
    )
    iota = np.ascontiguousarray(
        np.tile(np.arange(MAXCN, dtype=np.float32), (COUT, 1))
    )
    ident = np.eye(COUT, dtype=np.float32)
    shards = x.reshape(NCORES, BL, CIN, H, W)
    in_maps = [
        {
            "xs": shards[c],
            "wt": w,
            "bias": b,
            "cn": cnv,
            "iota": iota,
            "ident": ident,
        }
        for c in range(NCORES)
    ]
    return in_maps


def _get_runner(mode=_MODE):
    """Build + jit once; reuse across kernel() calls."""
    if mode in _RUNNER_CACHE:
        return _RUNNER_CACHE[mode]

    import jax
    from jax.experimental.shard_map import shard_map
    from jax.sharding import Mesh, PartitionSpec

    import concourse.mybir as mybir
    from concourse.bass2jax import (
        _bass_exec_p,
        install_neuronx_cc_hook,
        partition_id_tensor,
    )

    nc = build_nc(mode)
    install_neuronx_cc_hook()

    partition_name = nc.partition_id_tensor.name if nc.partition_id_tensor else None
    in_names = []
    out_names = []
    out_avals = []
    zero_shapes = []
    for alloc in nc.m.functions[0].allocations:
        if not isinstance(alloc, mybir.MemoryLocationSet):
            continue
        name = alloc.memorylocations[0].name
        if alloc.kind == "ExternalInput":
            if name != partition_name:
                in_names.append(name)
        elif alloc.kind == "ExternalOutput":
            out_names.append(name)
            shape = tuple(alloc.tensor_shape)
            dtype = mybir.dt.np(alloc.dtype)
            out_avals.append(jax.core.ShapedArray(shape, dtype))
            zero_shapes.append((shape, dtype))
    n_params = len(in_names)
    n_outs = len(out_names)
    all_names = in_names + out_names
    if partition_name is not None:
        all_names = all_names + [partition_name]

    def _body(*args):
        operands = list(args)
        if partition_name is not None:
            operands.append(partition_id_tensor())
        outs = _bass_exec_p.bind(
            *operands,
            out_avals=tuple(out_avals),
            in_names=tuple(all_names),
            out_names=tuple(out_names),
            lowering_input_output_aliases=(),
            sim_require_finite=True,
            sim_require_nnan=True,
            nc=nc,
        )
        return tuple(outs)

    devices = jax.devices()[:NCORES]
    mesh = Mesh(np.asarray(devices), ("core",))
    in_specs = (PartitionSpec("core"),) * (n_params + n_outs)
    out_specs = (PartitionSpec("core"),) * n_outs
    sharded = jax.jit(
        shard_map(
            _body, mesh=mesh, in_specs=in_specs, out_specs=out_specs, check_rep=False
        ),
        donate_argnums=tuple(range(n_params, n_params + n_outs)),
        keep_unused=True,
    )

    def run(in_maps):
        concat_in = [
            np.concatenate([np.asarray(in_maps[c][nm]) for c in range(NCORES)], axis=0)
            for nm in in_names
        ]
        concat_zeros = [
            np.zeros((NCORES * s[0],) + tuple(s[1:]), dt) for (s, dt) in zero_shapes
        ]
        out_arrs = sharded(*concat_in, *concat_zeros)
        # bf16 mode stores rounded outputs; widening to f32 is exact
        outv = np.asarray(out_arrs[0]).astype(np.float32)
        return outv.reshape(NCORES, BL, COUT, HO, WO)

    _RUNNER_CACHE[mode] = run
    return run


def kernel(x, weights, bias, connect_nums):
    run = _get_runner()
    in_maps = _make_inputs(x, weights, bias, connect_nums)
    outs = run(in_maps)
    return np.ascontiguousarray(outs.reshape(B, COUT, HO, WO))


# revision 7
# speedup vs baseline: 1.4543x; 1.2553x over previous
"""Bass/Tile Trainium2 kernel for masked-bank BatchConv2D.

Math (matches the reference nn.Module):
    mask[o, j]   = j < connect_nums[o]                       (j in [0, 64))
    kdense[o, c] = sum_{j : j%32==c} weights[o, j] * mask[o, j]   -> [64, 32, 3, 3]
    out          = conv2d(x, kdense, VALID) + bias[None]          -> [B, 64, 126, 126]

Strategy: data-parallel over batch (8 cores x 4 images). Per core, conv is
computed as 3 accumulating matmuls (one per kernel-column dx) with the
contraction dim packed as (dy, c) = 96 partitions. Input-image chunks are
replicated on-chip into 3 row-shifted partition blocks (X3 tile, built by
one HBM load + two SBUF->SBUF shift DMAs).

The kernel is SDMA-engine bound (16 engines/core, ~27 GB/s each), so the
main loop minimizes bytes through the DMA fabric:
  - operands and PE stream in bf16 (PE column-pair tiling: psum lower half
    = image A, upper half = image B, same output rows -> both halves share
    identical bias rows and stores are fully contiguous per partition)
  - bias is read from HBM once ([64, HO*WO] f32) and duplicated to
    partitions 64:128 by an on-chip SBUF->SBUF copy (fabric, not HBM)
  - outputs are stored as bf16 (the rounding happens on-device in the DVE
    psum-evacuation add; the host only widens bf16->f32, which is exact)
  - stores are one DMA per (image, 32-row chunk): [64 part, 8 KB contig]

Modes (BASS_CONV_MODE): "bf16" (default) as above; "f32" exact fp32 with
the same image-paired layout (f32 stores); "f32r" streams fp32 through the
PE's single-pass FP32R mode (~tf32, rel err ~2e-4) with a serial 64-part
psum layout (ISA: f32r matmul dst must start at partition 0, so no column
pairing).
"""

import os
import sys

for _p in ("/opt/trn_rl_repo",):
    if os.path.isdir(_p) and _p not in sys.path:
        sys.path.append(_p)

import numpy as np

# Problem dims (hardcoded per contract)
B, CIN, COUT = 32, 32, 64
H, W = 128, 128
KH = KW = 3
HO = WO = 126
MAXCN = 64
NCORES = 8
BL = B // NCORES  # local batch per core

# chunks of output rows per image: (x_row_start, n_x_rows, out_row_start, n_out_rows)
CHUNKS = [(0, 34, 0, 32), (32, 34, 32, 32), (64, 34, 64, 32), (96, 32, 96, 30)]
X3W = 34 * W  # x3 tile free size (elements)

_MODE = os.environ.get("BASS_CONV_MODE", "bf16")

_RUNNER_CACHE = {}


def _split_waits(nc, mybir, maxw=1):
    """This walrus build only accepts one sem-wait per instruction; hoist
    extra waits onto preceding NoOps on the same engine."""
    for f in nc.m.functions:
        for bb in f.blocks:
            newlist = []
            for inst in bb.instructions:
                si = inst.sync_info
                waits = list(si.on_wait) if si and si.on_wait else []
                if len(waits) > maxw:
                    chunks = [waits[i : i + maxw] for i in range(0, len(waits), maxw)]
                    for ci, ch in enumerate(chunks[:-1]):
                        nop = mybir.InstNoOp(
                            name=f"{inst.name}-ws{ci}", ins=[], outs=[]
                        )
                        nop.engine = inst.engine
                        nop.sync_info = mybir.SyncInfo(on_wait=list(ch), on_update=[])
                        newlist.append(nop)
                    si.on_wait = chunks[-1]
                newlist.append(inst)
            bb.instructions = newlist


def build_nc(mode=_MODE, split_waits=True):
    import concourse.bass as bass
    import concourse.mybir as mybir
    from concourse.tile import TileContext

    f32 = mybir.dt.float32
    i32 = mybir.dt.int32
    if mode == "bf16":
        mmdt = mybir.dt.bfloat16
    elif mode == "f32r":
        mmdt = mybir.dt.float32r
    else:
        mmdt = f32
    # storage dtype of matmul operand tiles: the BIR verifier requires fp32r
    # matmul operands to be *produced* as float32r, so the x3/lhsT tiles are
    # declared float32r and the copies into them perform the rounding.
    stdt = mmdt if mode in ("bf16", "f32r") else f32
    # DRAM output dtype: bf16 mode stores rounded outputs (host widens).
    outdt = mybir.dt.bfloat16 if mode == "bf16" else f32

    # f32r matmuls cannot target psum partitions 64:128 (ISA: dst partition
    # must be 0 for 4-byte non-exact modes), so f32r runs the "serial"
    # layout: one [64, N] psum tile at base 0 per output row-tile. bf16/f32
    # run the "paired" layout: two images concurrently via PE column
    # tiling (psum halves 0:64 / 64:128, same output rows).
    paired = mode != "f32r"

    nc = bass.Bass()
    xs = nc.declare_dram_parameter("xs", [BL, CIN, H, W], f32, isOutput=False)
    wt = nc.declare_dram_parameter("wt", [COUT, MAXCN * 9], f32, isOutput=False)
    bias = nc.declare_dram_parameter("bias", [COUT, HO * WO], f32, isOutput=False)
    cn = nc.declare_dram_parameter("cn", [COUT, 1], i32, isOutput=False)
    iota = nc.declare_dram_parameter("iota", [COUT, MAXCN], f32, isOutput=False)
    ident = nc.declare_dram_parameter("ident", [COUT, COUT], f32, isOutput=False)
    out = nc.declare_dram_parameter("out", [BL, COUT, HO * WO], outdt, isOutput=True)

    # queue-mode SBUF allocator: freed prep-pool space is not immediately
    # reused by the x3 pool, so x3 loads don't inherit a WAR dependency on
    # the weight-prep chain
    # bias is kept on-chip in bf16 in bf16 mode (bias magnitude is ~0.1 of
    # the conv output, so its rounding is far below the output's own bf16
    # rounding); halves the bias bytes through the DMA fabric.
    biasdt = mybir.dt.bfloat16 if mode == "bf16" else f32

    with TileContext(nc, pool_alloc_mode="queue") as tc:
        with tc.tile_pool(name="const", bufs=1) as constp:
            # persistent tiles
            bias2 = constp.tile([128 if paired else 64, HO * WO], biasdt)
            lhsT = constp.tile([96, 3 * COUT], stdt)

            # ---- weight prep ----
            with (
                tc.tile_pool(name="prep", bufs=1) as prepp,
                tc.tile_pool(name="tps", bufs=3, space="PSUM") as tpsp,
            ):
                prep_dmas = []
                w_sb = prepp.tile([COUT, MAXCN * 9], f32)
                prep_dmas.append(nc.sync.dma_start(out=w_sb[:], in_=wt[:]))
                cn_i = prepp.tile([COUT, 1], i32)
                prep_dmas.append(nc.sync.dma_start(out=cn_i[:], in_=cn[:]))
                iota_sb = prepp.tile([COUT, MAXCN], f32)
                prep_dmas.append(nc.sync.dma_start(out=iota_sb[:], in_=iota[:]))
                ident_sb = prepp.tile([COUT, COUT], f32)
                prep_dmas.append(nc.sync.dma_start(out=ident_sb[:], in_=ident[:]))

                cn_f = prepp.tile([COUT, 1], f32)
                nc.vector.tensor_copy(out=cn_f[:], in_=cn_i[:])
                mask = prepp.tile([COUT, MAXCN], f32)
                # mask[o, j] = (j < cn[o]) -> 1.0 / 0.0
                nc.vector.tensor_scalar(
                    out=mask[:],
                    in0=iota_sb[:],
                    scalar1=cn_f[:],
                    scalar2=None,
                    op0=mybir.AluOpType.is_lt,
                )
                # replicate mask over the 9 (dy,dx) slots: mask9[o, j*9+k]
                mask9 = prepp.tile([COUT, MAXCN * 9], f32)
                m9v = mask9.rearrange("p (j k) -> p j k", k=9)
                for k in range(9):
                    nc.vector.tensor_copy(out=m9v[:, :, k], in_=mask[:])
                wm = prepp.tile([COUT, MAXCN * 9], f32)
                nc.vector.tensor_mul(out=wm[:], in0=w_sb[:], in1=mask9[:])
                # fold j and j+32 (same input channel): kd[o, (c, dy, dx)]
                kd = prepp.tile([COUT, CIN * 9], f32)
                nc.vector.tensor_add(
                    out=kd[:], in0=wm[:, 0 : CIN * 9], in1=wm[:, CIN * 9 : MAXCN * 9]
                )
                # reorder to (dx, dy, c) contiguous, then transpose per dx:
                # [64, (dy, c)] -> [96, 64]
                kd4 = kd.rearrange("p (c dy dx) -> p dx dy c", c=CIN, dy=3, dx=3)
                kdr = prepp.tile([COUT, CIN * 9], f32)
                kdr4 = kdr.rearrange("p (dx dy c) -> p dx dy c", c=CIN, dy=3, dx=3)
                for dx in range(3):
                    nc.vector.tensor_copy(out=kdr4[:, dx], in_=kd4[:, dx])
                for dx in range(3):
                    tp = tpsp.tile([96, COUT], f32)
                    nc.tensor.transpose(
                        out=tp[:],
                        in_=kdr[:, dx * 96 : (dx + 1) * 96],
                        identity=ident_sb[:],
                    )
                    nc.vector.tensor_copy(
                        out=lhsT[:, dx * COUT : (dx + 1) * COUT], in_=tp[:]
                    )

            if paired:
                # ---- main loop: image-paired column tiling ----
                # macro-steps: (image pair, row chunk); psum lower half is
                # image 2*pr, upper half image 2*pr+1, same output rows.
                msteps = [
                    (2 * pr, r0, nxr, oy0, nor)
                    for pr in range(BL // 2)
                    for (r0, nxr, oy0, nor) in CHUNKS
                ]
                with (
                    tc.tile_pool(name="x3", bufs=6) as x3p,
                    tc.tile_pool(name="ps", bufs=6, space="PSUM") as psp,
                    tc.tile_pool(name="ob", bufs=2) as obp,
                ):
                    x3_tiles = {}

                    # SBUF AXI port map: partitions 0:64 ride the even DMA
                    # ports, 64:128 the odd ones. The x3 tile is pinned to
                    # partitions 0:96 (K=96 matmul => PE row-tile position
                    # 0), so the shift chain is routed to balance the port
                    # groups: base load writes even, shift+2 writes odd,
                    # and block1 is built FROM block2 (read odd, write even)
                    # instead of from the base block (read+write even).
                    def issue_x3(mi):
                        biA, r0, nxr, oy0, nor = msteps[mi]
                        pair = []
                        for k in (0, 1):
                            x3 = x3p.tile(
                                [96, X3W], stdt, tag=f"x3{k}", name=f"x3_{mi}_{k}"
                            )
                            if mode == "bf16":
                                # dtype-cast load must ride SWDGE (gpsimd)
                                nc.gpsimd.dma_start(
                                    out=x3[0:32, 0 : nxr * W],
                                    in_=xs[biA + k, :, r0 : r0 + nxr, :],
                                )
                            else:
                                nc.sync.dma_start(
                                    out=x3[0:32, 0 : nxr * W],
                                    in_=xs[biA + k, :, r0 : r0 + nxr, :],
                                )
                            nc.sync.dma_start(
                                out=x3[64:96, 0 : nor * W],
                                in_=x3[0:32, 2 * W : (nor + 2) * W],
                            )
                            nc.scalar.dma_start(
                                out=x3[32:64, 0:W], in_=x3[0:32, W : 2 * W]
                            )
                            nc.scalar.dma_start(
                                out=x3[32:64, W : nor * W],
                                in_=x3[64:96, 0 : (nor - 1) * W],
                            )
                            pair.append(x3)
                        x3_tiles[mi] = pair

                    issue_x3(0)
                    issue_x3(1)

                    # bias is loaded piecewise during the first image-pair's
                    # chunks so its 4 MB doesn't flood the DMA fabric at
                    # startup. Upper half (partitions 64:128) is an on-chip
                    # duplicate of the lower half (both psum halves cover the
                    # same output rows), built by SBUF->SBUF fabric DMA.
                    def issue_bias(ci):
                        _, _, oy0, nor = CHUNKS[ci]
                        a, b = oy0 * WO, (oy0 + nor) * WO
                        if mode == "bf16":
                            nc.gpsimd.dma_start(
                                out=bias2[0:64, a:b], in_=bias[:, a:b]
                            )
                        else:
                            nc.sync.dma_start(
                                out=bias2[0:64, a:b], in_=bias[:, a:b]
                            )
                        nc.sync.dma_start(
                            out=bias2[64:128, a:b], in_=bias2[0:64, a:b]
                        )

                    for mi, (biA, r0, nxr, oy0, nor) in enumerate(msteps):
                        if mi < len(CHUNKS):
                            issue_bias(mi)
                        if mi + 2 < len(msteps):
                            issue_x3(mi + 2)
                        x3A, x3B = x3_tiles.pop(mi)
                        xvA = x3A.rearrange("p (r c) -> p r c", c=W)
                        xvB = x3B.rearrange("p (r c) -> p r c", c=W)

                        # row-tiles of <=4 output rows; same tile index on
                        # both psum halves (image A lower, image B upper)
                        ntiles = (nor + 3) // 4
                        ob = obp.tile([128, 8 * 4 * WO], outdt, tag="ob")
                        t0 = 0
                        while t0 < ntiles:
                            g = min(4, ntiles - t0)
                            tss = []
                            for ti in range(t0, t0 + g):
                                nr = min(4, nor - 4 * ti)
                                ps = psp.tile([128, 4 * WO], f32, tag="ps")
                                tss.append((ti, nr, ps))
                            for dx in range(3):
                                lw = lhsT[:, dx * COUT : (dx + 1) * COUT]
                                for (ti, nr, ps) in tss:
                                    yl = 4 * ti
                                    N = nr * WO
                                    nc.tensor.matmul(
                                        ps[0:64, 0:N],
                                        lhsT=lw,
                                        rhs=xvA[:, yl : yl + nr, dx : dx + WO],
                                        start=(dx == 0),
                                        stop=(dx == 2),
                                        skip_group_check=True,
                                    )
                                    nc.tensor.matmul(
                                        ps[64:128, 0:N],
                                        lhsT=lw,
                                        rhs=xvB[:, yl : yl + nr, dx : dx + WO],
                                        start=(dx == 0),
                                        stop=(dx == 2),
                                        skip_group_check=True,
                                    )
                            # fused +bias psum evacuation; both halves read
                            # the same bias rows (upper = on-chip duplicate)
                            for (ti, nr, ps) in tss:
                                yg = oy0 + 4 * ti
                                N = nr * WO
                                nc.vector.tensor_add(
                                    out=ob[:, 4 * ti * WO : 4 * ti * WO + N],
                                    in0=ps[:, 0:N],
                                    in1=bias2[:, yg * WO : yg * WO + N],
                                )
                            t0 += g
                        # one contiguous store per image: [64, nor*WO]
                        nc.scalar.dma_start(
                            out=out[biA, :, oy0 * WO : (oy0 + nor) * WO],
                            in_=ob[0:64, 0 : nor * WO],
                        )
                        nc.scalar.dma_start(
                            out=out[biA + 1, :, oy0 * WO : (oy0 + nor) * WO],
                            in_=ob[64:128, 0 : nor * WO],
                        )
            else:
                # ---- main loop: serial layout (f32r) ----
                chunks = [
                    (bi, r0, nxr, oy0, nor)
                    for bi in range(BL)
                    for (r0, nxr, oy0, nor) in CHUNKS
                ]
                with (
                    tc.tile_pool(name="x3", bufs=4) as x3p,
                    tc.tile_pool(name="ps", bufs=8, space="PSUM") as psp,
                    tc.tile_pool(name="ob", bufs=2) as obp,
                ):
                    x3_tiles = {}

                    def issue_x3s(ci):
                        bi, r0, nxr, oy0, nor = chunks[ci]
                        x3 = x3p.tile([96, X3W], stdt, tag="x3", name=f"x3_{ci}")
                        xin = xs[bi, :, r0 : r0 + nxr, :]
                        if mode == "f32r":
                            xin = xin.bitcast(mmdt)
                        nc.sync.dma_start(out=x3[0:32, 0 : nxr * W], in_=xin)
                        nc.scalar.dma_start(
                            out=x3[32:64, 0 : nor * W],
                            in_=x3[0:32, W : (nor + 1) * W],
                        )
                        nc.gpsimd.dma_start(
                            out=x3[64:96, 0 : nor * W],
                            in_=x3[0:32, 2 * W : (nor + 2) * W],
                        )
                        x3_tiles[ci] = x3

                    issue_x3s(0)
                    issue_x3s(1)

                    def issue_bias_s(ci):
                        _, _, _, oy0, nor = chunks[ci]
                        a, b = oy0 * WO, (oy0 + nor) * WO
                        nc.gpsimd.dma_start(out=bias2[0:64, a:b], in_=bias[:, a:b])

                    for ci, (bi, r0, nxr, oy0, nor) in enumerate(chunks):
                        if ci < len(CHUNKS):
                            issue_bias_s(ci)
                        if ci + 2 < len(chunks):
                            issue_x3s(ci + 2)
                        x3 = x3_tiles.pop(ci)
                        x3v = x3.rearrange("p (r c) -> p r c", c=W)
                        # serial layout: 4-row tiles, dx-outer over groups
                        # of 8 tiles (weights change every 8 matmuls)
                        ntiles = (nor + 3) // 4
                        t0 = 0
                        while t0 < ntiles:
                            g = min(8, ntiles - t0)
                            tss = []
                            for ti in range(t0, t0 + g):
                                nr = min(4, nor - 4 * ti)
                                ps = psp.tile([64, 4 * WO], f32, tag="ps")
                                tss.append((ti, nr, ps))
                            for dx in range(3):
                                lw = lhsT[:, dx * COUT : (dx + 1) * COUT]
                                for (ti, nr, ps) in tss:
                                    yl = 4 * ti
                                    nc.tensor.matmul(
                                        ps[:, 0 : nr * WO],
                                        lhsT=lw,
                                        rhs=x3v[:, yl : yl + nr, dx : dx + WO],
                                        start=(dx == 0),
                                        stop=(dx == 2),
                                        skip_group_check=True,
                                    )
                            ob = obp.tile([64, 8 * 4 * WO], f32, tag="ob")
                            rows = 0
                            for gi, (ti, nr, ps) in enumerate(tss):
                                yg = oy0 + 4 * ti
                                N = nr * WO
                                o0 = gi * 4 * WO
                                nc.vector.tensor_add(
                                    out=ob[:, o0 : o0 + N],
                                    in0=ps[:, 0:N],
                                    in1=bias2[0:64, yg * WO : yg * WO + N],
                                )
                                rows += nr
                            yg0 = oy0 + 4 * t0
                            if rows == 4 * g:
                                nc.scalar.dma_start(
                                    out=out[bi, :, yg0 * WO : (yg0 + rows) * WO],
                                    in_=ob[:, 0 : rows * WO],
                                )
                            else:
                                # ragged tail: last tile shorter; store
                                # full tiles in one DMA, tail separately
                                nf = rows - tss[-1][1]
                                nc.scalar.dma_start(
                                    out=out[bi, :, yg0 * WO : (yg0 + nf) * WO],
                                    in_=ob[:, 0 : nf * WO],
                                )
                                nc.scalar.dma_start(
                                    out=out[
                                        bi,
                                        :,
                                        (yg0 + nf) * WO : (yg0 + rows) * WO,
                                    ],
                                    in_=ob[
                                        :,
                                        (g - 1) * 4 * WO : (g - 1) * 4 * WO
                                        + tss[-1][1] * WO,
                                    ],
                                )
                            t0 += g

    if split_waits:
        _split_waits(nc, mybir)
    return nc


def _make_inputs(x, weights, bias, connect_nums):
    """Host-side reshapes only (no input-dependent compute)."""
    x = np.ascontiguousarray(np.asarray(x, dtype=np.float32))
    w = np.ascontiguousarray(
        np.asarray(weights, dtype=np.float32).reshape(COUT, MAXCN * 9)
    )
    b = np.ascontiguousarray(np.asarray(bias, dtype=np.float32).reshape(COUT, HO * WO))
    cnv = np.ascontiguousarray(
        np.asarray(connect_nums, dtype=np.int32).reshape(COUT, 1)
    )
    iota = np.ascontiguousarray(
        np.tile(np.arange(MAXCN, dtype=np.float32), (COUT, 1))
    )
    ident = np.eye(COUT, dtype=np.float32)
    shards = x.reshape(NCORES, BL, CIN, H, W)
    in_maps = [
        {
            "xs": shards[c],
            "wt": w,
            "bias": b,
            "cn": cnv,
            "iota": iota,
            "ident": ident,
        }
        for c in range(NCORES)
    ]
    return in_maps


def _get_runner(mode=_MODE):
    """Build + jit once; reuse across kernel() calls."""
    if mode in _RUNNER_CACHE:
        return _RUNNER_CACHE[mode]

    import jax
    from jax.experimental.shard_map import shard_map
    from jax.sharding import Mesh, PartitionSpec

    import concourse.mybir as mybir
    from concourse.bass2jax import (
        _bass_exec_p,
        install_neuronx_cc_hook,
        partition_id_tensor,
    )

    nc = build_nc(mode)
    install_neuronx_cc_hook()

    partition_name = nc.partition_id_tensor.name if nc.partition_id_tensor else None
    in_names = []
    out_names = []
    out_avals = []
    zero_shapes = []
    for alloc in nc.m.functions[0].allocations:
        if not isinstance(alloc, mybir.MemoryLocationSet):
            continue
        name = alloc.memorylocations[0].name
        if alloc.kind == "ExternalInput":
            if name != partition_name:
                in_names.append(name)
        elif alloc.kind == "ExternalOutput":
            out_names.append(name)
            shape = tuple(alloc.tensor_shape)
            dtype = mybir.dt.np(alloc.dtype)
            out_avals.append(jax.core.ShapedArray(shape, dtype))
            zero_shapes.append((shape, dtype))
    n_params = len(in_names)
    n_outs = len(out_names)
    all_names = in_names + out_names
    if partition_name is not None:
        all_names = all_names + [partition_name]

    def _body(*args):
        operands = list(args)
        if partition_name is not None:
            operands.append(partition_id_tensor())
        outs = _bass_exec_p.bind(
            *operands,
            out_avals=tuple(out_avals),
            in_names=tuple(all_names),
            out_names=tuple(out_names),
            lowering_input_output_aliases=(),
            sim_require_finite=True,
            sim_require_nnan=True,
            nc=nc,
        )
        return tuple(outs)

    devices = jax.devices()[:NCORES]
    mesh = Mesh(np.asarray(devices), ("core",))
    in_specs = (PartitionSpec("core"),) * (n_params + n_outs)
    out_specs = (PartitionSpec("core"),) * n_outs
    sharded = jax.jit(
        shard_map(
            _body, mesh=mesh, in_specs=in_specs, out_specs=out_specs, check_rep=False
        ),
        donate_argnums=tuple(range(n_params, n_params + n_outs)),
        keep_unused=True,
    )

    def run(in_maps):
        concat_in = [
            np.concatenate([np.asarray(in_maps[c][nm]) for c in range(NCORES)], axis=0)
            for nm in in_names
        ]
        concat_zeros = [
            np.zeros((NCORES * s[0],) + tuple(s[1:]), dt) for (s, dt) in zero_shapes
        ]
        out_arrs = sharded(*concat_in, *concat_zeros)
        # bf16 mode stores rounded outputs; widening to f32 is exact
        outv = np.asarray(out_arrs[0]).astype(np.float32)
        return outv.reshape(NCORES, BL, COUT, HO, WO)

    _RUNNER_CACHE[mode] = run
    return run


def kernel(x, weights, bias, connect_nums):
    run = _get_runner()
    in_maps = _make_inputs(x, weights, bias, connect_nums)
    outs = run(in_maps)
    return np.ascontiguousarray(outs.reshape(B, COUT, HO, WO))
